# revision 4
# baseline (speedup 1.0000x reference)
"""Trainium2 Bass kernel for nn_CoulombPotential (PhysNet-attenuated Coulomb energy).

Algorithm
---------
  per_system[s] = KE * sum_{pairs p: i<j, sys(i)=s} q[i] q[j] chi(d_p)
  chi(d) = phi(2d)/sqrt(d^2+1) + (1-phi(2d))/d,  phi = PhysNet switching fn.

Sharding / host marshalling (no float arithmetic on host — only data movement):
  * drop masked (i>=j) pairs, group pairs by system (sys[idx_i]; sys is sorted
    over atoms), assign a contiguous block of 128 systems to each of 8 cores,
  * within a core, each system's pairs are padded to whole 512-slot rows, laid
    out as [640, 2048] f32 streams (5 tiles of [128, 2048] = 4 sub-rows of 512),
  * charge values for both endpoints are laid alongside as streams (gather is
    pure data movement), plus a 0/1 row->system selector for the PE.

Device (all arithmetic): chi pipeline on ACT+DVE, per-row sums on DVE, the
rows->systems segment reduction as 0/1-selector matmuls accumulated in PSUM on
the PE, and the final KE scale.  Core outputs are disjoint [128]-system slices;
the host only concatenates them.
"""
import functools

import numpy as np

import concourse.bacc as bacc
import concourse.bass_utils as bass_utils
import concourse.mybir as mybir
import concourse.tile as tile

F32 = mybir.dt.float32
AF = mybir.ActivationFunctionType
OP = mybir.AluOpType

KE = 138.96
N_CORES = 8
S_TOTAL = 1024
SYS_PER_CORE = S_TOTAL // N_CORES  # 128

PART = 128          # SBUF partitions
ROW = 512           # slots per logical row (system padding granularity)
T = 2048            # free dim per tile (= 4 sub-rows)
SUB = T // ROW      # sub-rows per partition per tile
TPC = 5             # tiles per core
ROWS_PER_TILE = PART * SUB          # 512 global rows per tile
ROWS_TOT = TPC * ROWS_PER_TILE      # 2560 rows per core
SLOTS = ROWS_TOT * ROW              # 1,310,720 slots per core


@functools.lru_cache(maxsize=2)
def _build_nc(repeat=0):
    """repeat=0: straight-line kernel.  repeat=R>0: wrap the body in a
    hardware For_i loop running it R times (identical result; used by the
    test harness to measure per-iteration device time via slope)."""
    nc = bacc.Bacc("TRN2", target_bir_lowering=False, debug=False,
                   enable_asserts=False, num_devices=N_CORES)
    d_in = nc.dram_tensor("d_in", [TPC * PART, T], F32, kind="ExternalInput")
    qi_in = nc.dram_tensor("qi_in", [TPC * PART, T], F32, kind="ExternalInput")
    qj_in = nc.dram_tensor("qj_in", [TPC * PART, T], F32, kind="ExternalInput")
    m_in = nc.dram_tensor("m_in", [TPC * PART, SUB * PART], F32,
                          kind="ExternalInput")
    out = nc.dram_tensor("out", [PART, 1], F32, kind="ExternalOutput")

    with tile.TileContext(nc) as tc:
        with (
            tc.tile_pool(name="io", bufs=2) as io,
            tc.tile_pool(name="tmp", bufs=2) as tmp,
            tc.tile_pool(name="acc", bufs=1) as acc,
            tc.tile_pool(name="psum", bufs=1, space="PSUM") as psp,
        ):
            ps = psp.tile([PART, 1], F32)

            def body():
                for t in range(TPC):
                    rs = slice(t * PART, (t + 1) * PART)
                    d = io.tile([PART, T], F32, tag="d")
                    qi = io.tile([PART, T], F32, tag="qi")
                    qj = io.tile([PART, T], F32, tag="qj")
                    mt = io.tile([PART, SUB, PART], F32, tag="mt")
                    nc.sync.dma_start(d[:], d_in[rs, :])
                    nc.sync.dma_start(qi[:], qi_in[rs, :])
                    nc.sync.dma_start(qj[:], qj_in[rs, :])
                    nc.sync.dma_start(mt[:], m_in[rs, :])

                    b1 = tmp.tile([PART, T], F32, tag="b1")
                    b2 = tmp.tile([PART, T], F32, tag="b2")
                    b3 = tmp.tile([PART, T], F32, tag="b3")
                    b4 = tmp.tile([PART, T], F32, tag="b4")
                    rsum = tmp.tile([PART, SUB], F32, tag="rsum")

                    # b1 = d^2 ; b4 = d^3 ; b1 <- sqrt(d^2+1) ; b1 <- rsqrt
                    nc.scalar.activation(b1[:], d[:], AF.Square)
                    nc.vector.tensor_tensor(b4[:], b1[:], d[:], OP.mult)
                    nc.scalar.activation(b1[:], b1[:], AF.Sqrt, bias=1.0, scale=1.0)
                    nc.vector.reciprocal(b1[:], b1[:])
                    # b2 = 1/d
                    nc.vector.reciprocal(b2[:], d[:])
                    # phi = relu(1 - ((192 d - 240) d + 80) d^3)   (in b3)
                    nc.vector.tensor_scalar(b3[:], d[:], 192.0, -240.0, OP.mult, OP.add)
                    nc.vector.tensor_tensor(b3[:], b3[:], d[:], OP.mult)
                    nc.vector.tensor_scalar(b3[:], b3[:], 80.0, None, OP.add)
                    nc.vector.tensor_tensor(b3[:], b3[:], b4[:], OP.mult)
                    nc.scalar.activation(b3[:], b3[:], AF.Relu, bias=1.0, scale=-1.0)
                    # chi = 1/d + phi*(1/sqrt(d^2+1) - 1/d)   (in b1)
                    nc.vector.tensor_tensor(b1[:], b1[:], b2[:], OP.subtract)
                    nc.vector.tensor_tensor(b1[:], b3[:], b1[:], OP.mult)
                    nc.vector.tensor_tensor(b1[:], b1[:], b2[:], OP.add)
                    # e = qi*qj*chi ; rowsums over the 4 sub-rows of 512
                    nc.vector.tensor_tensor(b1[:], qj[:], b1[:], OP.mult)
                    nc.vector.tensor_tensor(b1[:], qi[:], b1[:], OP.mult)
                    nc.vector.tensor_reduce(
                        rsum[:], b1[:].rearrange("p (s r) -> p s r", s=SUB),
                        mybir.AxisListType.X, OP.add)
                    for n in range(SUB):
                        nc.tensor.matmul(ps[:], mt[:, n, :], rsum[:, n:n + 1],
                                         start=(t == 0 and n == 0),
                                         stop=(t == TPC - 1 and n == SUB - 1))

            if repeat > 0:
                with tc.For_i(0, repeat, 1):
                    body()
            else:
                body()
            res = acc.tile([PART, 1], F32, tag="res")
            nc.scalar.mul(res[:], ps[:], KE)
            nc.sync.dma_start(out[:], res[:])
    nc.compile()
    return nc


def _host_marshal(electrostatic_pair_indices, electrostatic_d_ij,
                  per_atom_charge, atomic_subsystem_indices):
    idx_i = np.asarray(electrostatic_pair_indices[0])
    idx_j = np.asarray(electrostatic_pair_indices[1])
    d = np.asarray(electrostatic_d_ij)[:, 0]
    q = np.asarray(per_atom_charge)[:, 0].astype(np.float32)
    sys_idx = np.asarray(atomic_subsystem_indices)

    keep = idx_i < idx_j
    ii = idx_i[keep]
    jj = idx_j[keep]
    dd = d[keep].astype(np.float32)
    seg = sys_idx[ii].astype(np.int64)

    order = np.argsort(seg, kind="stable")
    ii = ii[order]
    jj = jj[order]
    dd = dd[order]
    seg = seg[order]

    counts = np.bincount(seg, minlength=S_TOTAL)
    sys_start = np.concatenate([[0], np.cumsum(counts)])

    # The i<j mask keeps more pairs for low atom indices, so per-system pair
    # counts fall roughly linearly with system id; a contiguous block split
    # is badly imbalanced.  Serpentine-assign systems (by descending count)
    # to cores: balanced within ~1% and exactly 128 systems per core.
    order_sys = np.argsort(-counts, kind="stable")
    k = np.arange(S_TOTAL)
    block, within = k // N_CORES, k % N_CORES
    core_of_rank = np.where(block % 2 == 0, within, N_CORES - 1 - within)
    sys_to_core = np.empty(S_TOTAL, np.int64)
    sys_to_core[order_sys] = core_of_rank
    # local slot of each system within its core (order of assignment)
    sys_to_local = np.empty(S_TOTAL, np.int64)
    core_systems = np.empty((N_CORES, SYS_PER_CORE), np.int64)
    for c in range(N_CORES):
        mine = order_sys[core_of_rank == c]
        core_systems[c] = mine
        sys_to_local[mine] = np.arange(SYS_PER_CORE)

    # per-core row layout: each system padded to whole 512-slot rows
    rows_of_sys = -(-counts // ROW)               # global, by system id
    core_row_base = np.empty(S_TOTAL, np.int64)   # first row of sys in its core
    n_rows_core = np.empty(N_CORES, np.int64)
    for c in range(N_CORES):
        mine = core_systems[c]
        rb = np.concatenate([[0], np.cumsum(rows_of_sys[mine])])
        core_row_base[mine] = rb[:-1]
        n_rows_core[c] = rb[-1]
    assert n_rows_core.max() <= ROWS_TOT, n_rows_core
    assert int(counts.max()) <= ROWS_TOT * ROW

    dest_core = sys_to_core[seg]
    dest_slot = core_row_base[seg] * ROW + (np.arange(len(seg)) - sys_start[seg])

    in_maps = []
    for c in range(N_CORES):
        sel = dest_core == c
        dest = dest_slot[sel]
        dstream = np.ones(SLOTS, np.float32)
        qis = np.zeros(SLOTS, np.float32)
        qjs = np.zeros(SLOTS, np.float32)
        dstream[dest] = dd[sel]
        qis[dest] = q[ii[sel]]
        qjs[dest] = q[jj[sel]]

        # 0/1 selector: global row g (slot // ROW) -> local system slot
        row_sys = np.repeat(sys_to_local[core_systems[c]],
                            rows_of_sys[core_systems[c]])
        m = np.zeros((ROWS_TOT, SYS_PER_CORE), np.float32)
        m[np.arange(n_rows_core[c]), row_sys] = 1.0
        # row g = t*512 + p*4 + n  ->  [TPC, PART, SUB, 128] -> [TPC*PART, SUB*128]
        m = m.reshape(TPC, PART, SUB, SYS_PER_CORE).reshape(TPC * PART, SUB * SYS_PER_CORE)

        in_maps.append({
            "d_in": dstream.reshape(TPC * PART, T),
            "qi_in": qis.reshape(TPC * PART, T),
            "qj_in": qjs.reshape(TPC * PART, T),
            "m_in": np.ascontiguousarray(m),
        })
    return in_maps, core_systems


def kernel(electrostatic_pair_indices, electrostatic_d_ij, per_atom_charge,
           atomic_subsystem_indices, num_systems):
    assert int(num_systems) == S_TOTAL
    in_maps, core_systems = _host_marshal(
        electrostatic_pair_indices, electrostatic_d_ij,
        per_atom_charge, atomic_subsystem_indices)
    nc = _build_nc()
    res = bass_utils.run_bass_kernel_spmd(nc, in_maps,
                                          core_ids=list(range(N_CORES)))
    full = np.empty(S_TOTAL, np.float32)
    for c in range(N_CORES):
        full[core_systems[c]] = res.results[c]["out"][:, 0]
    return full[:, None]
